# revision 34
# baseline (speedup 1.0000x reference)
"""Trainium2 Bass kernel for additive-attention nn.Module.

Math: reference computes
    scores[b,i,j] = x[b,i,:]@W[0,:3] + key[b,j,:]@W[0,3:] + b0
    attn = softmax(scores, axis=j) ; out = attn @ value

softmax over j is shift-invariant, so the x- and bias-terms (constant in j)
cancel exactly: attn[b,i,j] = softmax_j(key[b,j,:]@W[0,3:]) independent of i.
Hence out[b,i,:] = sum_j p[b,j] * value[b,j,:]  (identical for every i).

Kernel (data-parallel over batch, 8 batches/core on 8 cores):
  1. sk[b,j] = key[b,j,:] . w_k             (DVE fused mul-add)
  2. e[b,:]  = exp(sk - max), s = sum(e)    (DVE reduce_max / ACT exp+sum)
  3. eT_il   = interleaved transpose of e   (PE): eT[q, jj*8+b] = e[b, 8q+jj]
     rb[q,b] = 1/s[b] on every partition    (PE ones@diag trick)
  4. sc[q,jj,:] = e[b,8q+jj]*value[b,8q+jj,:]  (scales split DVE/ACT;
     value loaded in its natural DRAM layout: partition q holds rows
     8q..8q+7 contiguously -> 2-8KB DMA packets)
  5. two tree-add levels on DVE, then two accumulating all-ones matmuls
     fuse the last level + partition-reduce + broadcast (PE, exact fp32)
  6. o_sb = bc * (1/s[b]) twice side by side (ACT), out[b] written as
     4 plain DMAs of (128,512) -> 2KB contiguous packets both sides
"""

import numpy as np
from contextlib import ExitStack

import concourse.bass as bass
import concourse.bacc as bacc
import concourse.mybir as mybir
from concourse import tile
from concourse.bass_utils import run_bass_kernel_spmd

B, S1, S2, DV = 64, 1024, 1024, 256
NCORES = 8
BPC = B // NCORES            # batches per core
NJ = S2 // 128               # j-chunks / row-interleave factor
NR = S1 // 128               # output row-repeats per partition
F32 = mybir.dt.float32

N_DVE_SCALES = 4             # scale ops per batch on DVE; rest on ACT

_compiled = {}


def _build_nc():
    nc = bacc.Bacc("TRN2", target_bir_lowering=False, debug=False,
                   num_devices=NCORES)

    key_d = nc.dram_tensor("key", [BPC, S2, 3], F32, kind="ExternalInput")
    val_d = nc.dram_tensor("value", [BPC, S2, DV], F32, kind="ExternalInput")
    wk_d = nc.dram_tensor("wkb", [BPC, 3], F32, kind="ExternalInput")
    ones_d = nc.dram_tensor("ones", [128, 128], F32, kind="ExternalInput")
    id_d = nc.dram_tensor("ident", [BPC, BPC], F32, kind="ExternalInput")
    out_d = nc.dram_tensor("out", [BPC, S1, DV], F32, kind="ExternalOutput")

    with tile.TileContext(nc) as tc, ExitStack() as ctx:
        const = ctx.enter_context(tc.tile_pool(name="const", bufs=1))
        sm = ctx.enter_context(tc.tile_pool(name="sm", bufs=1))
        vpool = ctx.enter_context(tc.tile_pool(name="v", bufs=8))
        apool = ctx.enter_context(tc.tile_pool(name="a", bufs=8))
        opool = ctx.enter_context(tc.tile_pool(name="o", bufs=8))
        ps_tp = ctx.enter_context(
            tc.tile_pool(name="ps_tp", bufs=2, space=bass.MemorySpace.PSUM))
        ps_rb = ctx.enter_context(
            tc.tile_pool(name="ps_rb", bufs=1, space=bass.MemorySpace.PSUM))
        ps_bc = ctx.enter_context(
            tc.tile_pool(name="ps_bc", bufs=5, space=bass.MemorySpace.PSUM))

        k_sb = sm.tile([BPC, S2 * 3], F32)
        k_src = key_d.ap().rearrange("b j f -> b (j f)")
        nc.sync.dma_start(k_sb[:, 0:1536], k_src[:, 0:1536])
        nc.sync.dma_start(k_sb[:, 1536:3072], k_src[:, 1536:3072])
        k3 = k_sb[:].rearrange("b (j f) -> b j f", f=3)

        wk_sb = const.tile([BPC, 3], F32)
        nc.sync.dma_start(wk_sb[:], wk_d[:])
        ones_sb = const.tile([128, 128], F32)
        nc.sync.dma_start(ones_sb[:], ones_d[:])
        id_sb = const.tile([BPC, BPC], F32)
        nc.sync.dma_start(id_sb[:], id_d[:])

        # all value DMAs issued up front: GpSimd takes the outer pieces,
        # Vector (idle until the key arrives) the middle piece of each batch
        v_tiles = []
        for b in range(BPC):
            v_sb = vpool.tile([128, NJ * DV], F32, tag="v_sb")
            v_src = val_d.ap()[b].rearrange("(q jj) d -> q (jj d)", q=128)
            if b < 2:
                cuts = (0, 512, 1024, 1536, 2048)
            else:
                cuts = (0, 1024, 2048)
            for lo, hi in zip(cuts[:-1], cuts[1:]):
                nc.gpsimd.dma_start(v_sb[:, lo:hi], v_src[:, lo:hi])
            v_tiles.append(v_sb)

        # sk = key . w_k  (3-term dot via fused mul-add)
        sk0 = sm.tile([BPC, S2], F32)
        sk1 = sm.tile([BPC, S2], F32)
        sk2 = sm.tile([BPC, S2], F32)
        nc.vector.tensor_scalar_mul(sk0[:], k3[:, :, 0], wk_sb[:, 0:1])
        nc.vector.scalar_tensor_tensor(
            sk1[:], k3[:, :, 1], wk_sb[:, 1:2], sk0[:],
            op0=mybir.AluOpType.mult, op1=mybir.AluOpType.add)
        nc.vector.scalar_tensor_tensor(
            sk2[:], k3[:, :, 2], wk_sb[:, 2:3], sk1[:],
            op0=mybir.AluOpType.mult, op1=mybir.AluOpType.add)

        # softmax numerator over j (free dim); normalization happens at the
        # very end via rb = 1/s broadcast (saves a full-width DVE pass)
        e = sm.tile([BPC, S2], F32)
        s = sm.tile([BPC, 1], F32)
        nc.scalar.activation(e[:], sk2[:], mybir.ActivationFunctionType.Exp,
                             bias=0.0, scale=1.0, accum_out=s[:])
        r = sm.tile([BPC, 1], F32)
        nc.vector.reciprocal(r[:], s[:])

        # interleaved transpose of the unnormalized weights:
        # eT[q, jj*BPC+b] = e[b, q*NJ+jj]
        e_il = e[:].rearrange("b (q jj) -> b jj q", jj=NJ)
        eT = sm.tile([128, NJ * BPC], F32)
        for jj in range(NJ):
            tp = ps_tp.tile([128, BPC], F32)
            nc.tensor.transpose(tp[:], e_il[:, jj, :], id_sb[:])
            nc.vector.tensor_copy(eT[:, jj * BPC:(jj + 1) * BPC], tp[:])

        # rb[q, b] = r[b] on all 128 partitions: ones(8,128).T @ (id * r)
        rdiag = sm.tile([BPC, BPC], F32)
        nc.vector.tensor_scalar_mul(rdiag[:], id_sb[:], r[:])
        rb_ps = ps_rb.tile([128, BPC], F32)
        nc.tensor.matmul(rb_ps[:], ones_sb[0:BPC, :], rdiag[:],
                         start=True, stop=True)
        rb = sm.tile([128, BPC], F32)
        nc.vector.tensor_copy(rb[:], rb_ps[:])

        for b in range(BPC):
            v_sb = v_tiles[b]
            # sc[q, jj, d] = e[b, 8q+jj] * value[b, 8q+jj, d]
            sc = apool.tile([128, NJ, DV], F32, tag="sc")
            for jj in range(NJ):
                scol = eT[:, jj * BPC + b:jj * BPC + b + 1]
                vin = v_sb[:, jj * DV:(jj + 1) * DV]
                if jj < N_DVE_SCALES:
                    nc.vector.tensor_scalar_mul(sc[:, jj, :], vin, scol)
                else:
                    nc.scalar.mul(sc[:, jj, :], vin, scol)

            # two tree-add levels (DVE); last level folds into the matmuls
            nc.vector.tensor_add(sc[:, 0:4, :], sc[:, 0:4, :], sc[:, 4:8, :])
            nc.vector.tensor_add(sc[:, 0:2, :], sc[:, 0:2, :], sc[:, 2:4, :])

            # fused last tree level + partition-reduce + broadcast (exact):
            # bc[m,d] = sum_q (sc[q,0,d] + sc[q,1,d])
            bc_ps = ps_bc.tile([128, DV], F32)
            nc.tensor.matmul(bc_ps[:], ones_sb[:], sc[:, 0, :],
                             start=True, stop=False)
            nc.tensor.matmul(bc_ps[:], ones_sb[:], sc[:, 1, :],
                             start=False, stop=True)

            # normalize while copying out of PSUM; two copies side by side
            # give 2KB contiguous source rows
            o_sb = opool.tile([128, 2 * DV], F32)
            bc2 = bc_ps[:].rearrange("q (a d) -> q a d", a=1).broadcast_to(
                (128, 2, DV))
            nc.scalar.mul(o_sb[:].rearrange("q (t d) -> q t d", t=2), bc2,
                          rb[:, b:b + 1])

            # out[b]: 4 plain DMAs of (128, 512); both sides 2KB contiguous
            ov = out_d.ap()[b].rearrange("(q rr) d -> q rr d", q=128)
            for g in range(4):
                dst = ov[:, 2 * g:2 * g + 2, :].rearrange("q t d -> q (t d)")
                nc.sync.dma_start(dst, o_sb[:])

    nc.compile()
    return nc


def _get_nc():
    if "nc" not in _compiled:
        _compiled["nc"] = _build_nc()
    return _compiled["nc"]


def _make_in_maps(key, value, W):
    key = np.ascontiguousarray(np.asarray(key, dtype=np.float32))
    value = np.ascontiguousarray(np.asarray(value, dtype=np.float32))
    W = np.asarray(W, dtype=np.float32)
    wkb = np.ascontiguousarray(np.tile(W[0, 3:].reshape(1, 3), (BPC, 1)))
    ones = np.ones((128, 128), dtype=np.float32)
    ident = np.eye(BPC, dtype=np.float32)
    in_maps = []
    for c in range(NCORES):
        lo, hi = c * BPC, (c + 1) * BPC
        in_maps.append({
            "key": np.ascontiguousarray(key[lo:hi]),
            "value": np.ascontiguousarray(value[lo:hi]),
            "wkb": wkb,
            "ones": ones,
            "ident": ident,
        })
    return in_maps


def kernel(x, key, value, W, b):
    nc = _get_nc()
    in_maps = _make_in_maps(key, value, W)
    res = run_bass_kernel_spmd(nc, in_maps, core_ids=list(range(NCORES)))
    return np.concatenate([r["out"] for r in res.results], axis=0)


def kernel_traced(x, key, value, W, b, **spmd_kwargs):
    """Like kernel() but returns (output, BassKernelResults) — for test.py."""
    nc = _get_nc()
    in_maps = _make_in_maps(key, value, W)
    res = run_bass_kernel_spmd(nc, in_maps, core_ids=list(range(NCORES)),
                               **spmd_kwargs)
    return np.concatenate([r["out"] for r in res.results], axis=0), res
